# revision 19
# baseline (speedup 1.0000x reference)
"""Trainium2 Bass kernel for DynamicConv2d (MoE-routed per-sample conv).

Data-parallel over batch: 32 samples -> 8 NeuronCores, 4 samples each.

The 3x3 conv runs as Winograd F(2,3) along H: the host pre-transforms the
weight bank with G = [[1,0,0],[.5,.5,.5],[.5,-.5,.5],[0,0,1]] (4 Winograd
components x 3 w-taps = 12 taps, bf16, matmul-stationary layout), the
device transforms the padded input rows with B^T on the DVE, the PE
contracts (component, cin, w-tap) into the m-domain with 2/3 the MACs of
direct conv, and the inverse transform A^T (4 tensor-ops per section)
produces output rows straight into the store tiles -- it replaces the
plain PSUM-drain copies, so the only extra DVE cost is the input
transform.

Per sample: load x (row-halves, ScalarE f32->bf16 pad-convert with
accum_out channel sums), router MLP + softmax, mix the 4 banks into
per-sample 12-tap weights (bf16 chain: ScalarE scaled-copy + DVE fused
multiply-adds), Winograd-transform the input, then 672 matmuls of
[128x128]x[128,112] accumulating f32 in PSUM.  Sections of 3 row-batches
(12 output rows) keep 2x3 PSUM banks in flight; the emission order keeps
next-sample work (input transform, mixing) out of the DVE FIFO path of
the current sample's PSUM inverse-transforms.
"""

import numpy as np
import ml_dtypes

import concourse.bass as bass
import concourse.tile as tile
from concourse import bacc, mybir
from concourse import bass_utils

F32 = mybir.dt.float32
BF16 = mybir.dt.bfloat16
AF = mybir.ActivationFunctionType
ALU = mybir.AluOpType
AX = mybir.AxisListType

B, CIN, H, W = 32, 256, 56, 56
COUT, KB, KK = 256, 4, 3
HID = 64
N_CORES = 8
BL = B // N_CORES          # samples per core
CICH = CIN // 128          # cin chunks
OCCH = COUT // 128         # cout chunks
NT = 12                    # winograd taps: 4 components x 3 w-taps
HP = H + 2                 # padded height (58)
WP = W + 2                 # padded width (58)
NG = H // 2                # winograd row-groups (28)
# sections: (first batch, n batches); batch = 2 row-groups = 4 output rows
SECTIONS = [(0, 3), (3, 3), (6, 3), (9, 3), (12, 2)]


def _emit_router(nc, pools, state, b):
    """x load (row-halves) + pad-convert + channel sums + router MLP +
    softmax for sample b.  Returns broadcast routing weights [128, KB]."""
    xstage_p, small_p, aux_psum = (
        pools["xstage"], pools["small"], pools["aux_psum"])
    x_ap = state["x_ap"]
    xpad = state["xpad"]
    fc1_wT, fc2_wT = state["fc1_wT"], state["fc2_wT"]
    fc1_b, fc2_b = state["fc1_b"], state["fc2_b"]

    par = b % 2
    HH = H // 2
    v4 = small_p.tile([128, 2 * CICH], F32, tag="v", name=f"v_{b}")
    xst = []
    for ci in range(CICH):
        t = xstage_p.tile([128, H * W], F32, tag="xs", name=f"xs_{b}_{ci}")
        for hh in range(2):
            nc.sync.dma_start(
                t[:, hh * HH * W:(hh + 1) * HH * W],
                x_ap[b, ci * 128:(ci + 1) * 128, hh * HH:(hh + 1) * HH]
                .rearrange("c h w -> c (h w)"))
        xst.append(t)
    for ci in range(CICH):
        tv = xst[ci].rearrange("c (h w) -> c h w", w=W)
        for hh in range(2):
            # f32 -> bf16 convert into the padded image (borders zeroed at
            # setup); accum_out gives partial channel sums for the router
            nc.scalar.activation(
                xpad[par][ci][:, 1 + hh * HH:1 + (hh + 1) * HH, 1:W + 1],
                tv[:, hh * HH:(hh + 1) * HH, :], AF.Copy,
                accum_out=v4[:, 2 * ci + hh:2 * ci + hh + 1])

    # router MLP: h = relu(fc1_w @ (v/3136) + b1); logits = h @ fc2_wT + b2
    psum_h = aux_psum.tile([HID, 1], F32, tag="aux", bufs=1, name=f"ph_{b}")
    for j in range(2 * CICH):
        nc.tensor.matmul(psum_h[:], fc1_wT[:, j // 2, :], v4[:, j:j + 1],
                         start=(j == 0), stop=(j == 2 * CICH - 1))
    h_sb = small_p.tile([HID, 1], F32, tag="h", name=f"h_{b}")
    nc.scalar.activation(h_sb[:], psum_h[:], AF.Relu, bias=fc1_b[:])

    psum_l = aux_psum.tile([1, KB], F32, tag="aux", bufs=1, name=f"pl_{b}")
    nc.tensor.matmul(psum_l[:], h_sb[:], fc2_wT[:])
    logit = small_p.tile([1, KB], F32, tag="lg", name=f"lg_{b}")
    nc.vector.tensor_add(logit[:], psum_l[:], fc2_b[:])

    # softmax over the 4 banks
    nmax = small_p.tile([1, 1], F32, tag="nm", name=f"nm_{b}")
    nc.vector.reduce_max(nmax[:], logit[:], axis=AX.X, negate=True)
    e_sb = small_p.tile([1, KB], F32, tag="e", name=f"e_{b}")
    s_sb = small_p.tile([1, 1], F32, tag="s", name=f"s_{b}")
    nc.scalar.activation(e_sb[:], logit[:], AF.Exp, bias=nmax[:, 0:1],
                         accum_out=s_sb[:])
    r_sb = small_p.tile([1, 1], F32, tag="r", name=f"r_{b}")
    nc.vector.reciprocal(r_sb[:], s_sb[:])
    a_sb = small_p.tile([1, KB], F32, tag="a", name=f"a_{b}")
    nc.vector.tensor_scalar_mul(a_sb[:], e_sb[:], r_sb[:, 0:1])
    # broadcast across partitions with a tiny ones-matmul
    psum_bc = aux_psum.tile([128, KB], F32, tag="aux", bufs=1, name=f"pb_{b}")
    nc.tensor.matmul(psum_bc[:], state["ones"][:], a_sb[:])
    a_bc = small_p.tile([128, KB], F32, tag="abc", name=f"abc_{b}")
    nc.vector.tensor_copy(a_bc[:], psum_bc[:])
    return a_bc


def _emit_input_tf(nc, pools, state, b):
    """Winograd B^T input transform along H: xpad -> xT[par] components."""
    xpad = state["xpad"]
    xT = state["xT"]
    par = b % 2
    for ci in range(CICH):
        v = xpad[par][ci].rearrange("p (g two) w -> p g two w", two=2)
        t = xT[par][ci]
        e0 = v[:, 0:NG, 0, :]       # padded rows 2g
        o0 = v[:, 0:NG, 1, :]       # rows 2g+1
        e1 = v[:, 1:NG + 1, 0, :]   # rows 2g+2
        o1 = v[:, 1:NG + 1, 1, :]   # rows 2g+3
        nc.vector.tensor_sub(t[:, :, 0, :], e0, e1)
        nc.vector.tensor_add(t[:, :, 1, :], o0, e1)
        nc.vector.tensor_sub(t[:, :, 2, :], e1, o0)
        nc.vector.tensor_sub(t[:, :, 3, :], o0, o1)


def _emit_mixing_oc(nc, pools, state, b, a_bc, oc, wdyn):
    """Bank mixing k0..k3 for one cout half: wd = sum_k a[k]*bank12.

    Each pass is split into 3 tap-slices (~0.6us DVE ops instead of one
    1.8us op): the mixing runs in 1x DVE mode, and coarse ops sitting in
    the DVE queue delay the PSUM inverse-transforms the PE is waiting on.
    Slice-major emission also completes wd tap-slices early, so the next
    sample's first matmuls (which only need taps 0-2) unblock sooner."""
    wacc_p, wdyn_p = pools["wacc"], pools["wdyn"]
    bank = state["bank"]
    NS = 3
    TS = NT // NS
    for ci in range(CICH):
        wa = wacc_p.tile([128, NT, 128], BF16, tag="wa",
                         name=f"wa_{b}_{ci}_{oc}")
        wd = wdyn_p.tile([128, NT, 128], BF16, tag="wd",
                         name=f"wd_{b}_{ci}_{oc}")
        for sl in range(NS):
            t0, t1 = sl * TS, (sl + 1) * TS
            nc.scalar.activation(wa[:, t0:t1, :], bank[ci][0][oc][:, t0:t1, :],
                                 AF.Copy, scale=a_bc[:, 0:1])
            for k in range(1, KB - 1):
                nc.vector.scalar_tensor_tensor(
                    wa[:, t0:t1, :], bank[ci][k][oc][:, t0:t1, :],
                    a_bc[:, k:k + 1], wa[:, t0:t1, :],
                    op0=ALU.mult, op1=ALU.add)
            nc.vector.scalar_tensor_tensor(
                wd[:, t0:t1, :], bank[ci][KB - 1][oc][:, t0:t1, :],
                a_bc[:, KB - 1:KB], wa[:, t0:t1, :],
                op0=ALU.mult, op1=ALU.add)
        wdyn[(ci, oc)] = wd


def _emit_sample_conv(nc, pools, state, b, wdyn, oc):
    """Winograd conv for one cout half; m-domain matmuls + A^T transform."""
    pyh_p, ysb_p, tft_p = pools["pyh_psum"], pools["ysb"], pools["tft"]
    y_ap = state["y_ap"]
    xT = state["xT"]
    par = b % 2
    for si, (b0, nb) in enumerate(SECTIONS):
        pyh = pyh_p.tile([128, 3, 4, 2, 64], F32, tag="pyh",
                         name=f"pyh_{b}_{oc}_{si}")
        for c in range(4):
            for ci in range(CICH):
                for dw in range(KK):
                    lhsT = wdyn[(ci, oc)][:, c * KK + dw, :]
                    st = (ci == 0 and dw == 0)
                    sp = (ci == CICH - 1 and dw == KK - 1)
                    for j in range(nb):
                        g0 = (b0 + j) * 2
                        rhs = xT[par][ci][:, g0:g0 + 2, c, dw:dw + W]
                        nc.tensor.matmul(pyh[:, j, c, :, 0:W], lhsT, rhs,
                                         start=st, stop=sp)
        # inverse transform A^T: y_even = m0+m1+m2, y_odd = m1-m2-m3.
        # DVE tensor_tensor may read only ONE operand from PSUM, so m1 is
        # first staged to SBUF by the ScalarE.
        t1 = tft_p.tile([128, 3, 2, W], F32, tag="t1", name=f"t1_{b}_{oc}_{si}")
        te = tft_p.tile([128, 3, 2, W], F32, tag="te", name=f"te_{b}_{oc}_{si}")
        to = tft_p.tile([128, 3, 2, W], F32, tag="to", name=f"to_{b}_{oc}_{si}")
        ysb = ysb_p.tile([128, 3, 2, 2, W], F32, tag="ysb",
                         name=f"ysb_{b}_{oc}_{si}")
        m = [pyh[:, 0:nb, c, :, 0:W] for c in range(4)]
        nc.scalar.activation(t1[:, 0:nb], m[1], AF.Copy)
        nc.vector.tensor_add(te[:, 0:nb], m[0], t1[:, 0:nb])
        nc.vector.tensor_add(ysb[:, 0:nb, :, 0, :], te[:, 0:nb], m[2])
        nc.vector.tensor_sub(to[:, 0:nb], t1[:, 0:nb], m[2])
        nc.vector.tensor_sub(ysb[:, 0:nb, :, 1, :], to[:, 0:nb], m[3])
        nc.sync.dma_start(
            y_ap[b, oc * 128:(oc + 1) * 128, 12 * si:12 * si + 4 * nb, :],
            ysb[:, 0:nb].rearrange("p j g q w -> p (j g q) w"))


def build_kernel(nc, tc, x_ap, bank_ap, fc1wT_ap, fc1b_ap, fc2wT_ap,
                 fc2b_ap, y_ap):
    const_p = tc.alloc_tile_pool(name="const", bufs=1)
    pools = {
        "xstage": tc.alloc_tile_pool(name="xstage", bufs=2),
        "wacc": tc.alloc_tile_pool(name="wacc", bufs=2),
        "wdyn": tc.alloc_tile_pool(name="wdyn", bufs=8),
        "small": tc.alloc_tile_pool(name="small", bufs=2),
        "ysb": tc.alloc_tile_pool(name="ysb", bufs=4),
        "tft": tc.alloc_tile_pool(name="tft", bufs=2),
    }
    pools["aux_psum"] = tc.alloc_tile_pool(name="aux_psum", bufs=1,
                                           space="PSUM")
    pools["pyh_psum"] = tc.alloc_tile_pool(name="pyh_psum", bufs=2,
                                           space="PSUM")

    # ---- constants -------------------------------------------------------
    ones = const_p.tile([1, 128], F32, name="ones")
    nc.vector.memset(ones[:], 1.0)

    # tiny router consts ride the Activation HWDGE queue
    fc1_wT = const_p.tile([128, CICH, HID], F32, name="fc1_wT")
    nc.scalar.dma_start(fc1_wT[:], fc1wT_ap)
    fc2_wT = const_p.tile([HID, KB], F32, name="fc2_wT")
    nc.scalar.dma_start(fc2_wT[:], fc2wT_ap)
    fc1_b = const_p.tile([HID, 1], F32, name="fc1_b")
    nc.scalar.dma_start(fc1_b[:], fc1b_ap.unsqueeze(1))
    fc2_b = const_p.tile([1, KB], F32, name="fc2_b")
    nc.scalar.dma_start(fc2_b[:], fc2b_ap.unsqueeze(0))

    # padded conv inputs (2 parities x 2 cin chunks); border-zero only
    xpad = [[const_p.tile([128, HP, WP], BF16, name=f"xpad_{p}_{ci}")
             for ci in range(CICH)] for p in range(2)]
    for p in range(2):
        for ci in range(CICH):
            t = xpad[p][ci]
            nc.vector.memset(t[:, 0, :], 0.0)
            nc.vector.memset(t[:, HP - 1, :], 0.0)
            nc.vector.memset(t[:, 1:HP - 1, 0], 0.0)
            nc.vector.memset(t[:, 1:HP - 1, WP - 1], 0.0)

    # winograd-transformed input [p, row-group, component, padded col]
    xT = [[const_p.tile([128, NG, 4, WP], BF16, name=f"xT_{p}_{ci}")
           for ci in range(CICH)] for p in range(2)]

    # ---- bank stream-in: host-side G-transform + transpose + bf16 -------
    bank = [[[const_p.tile([128, NT, 128], BF16, name=f"bank_{ci}_{k}_{oc}")
              for oc in range(OCCH)] for k in range(KB)]
            for ci in range(CICH)]

    state = {"x_ap": x_ap, "y_ap": y_ap, "xpad": xpad, "xT": xT,
             "ones": ones, "fc1_b": fc1_b, "fc2_b": fc2_b, "bank": bank,
             "fc1_wT": fc1_wT, "fc2_wT": fc2_wT}

    # ---- software-pipelined per-sample loop ------------------------------
    # x0's DMAs lead the SP queue, then bank chunks (cout-half 0 first).
    a_bc0 = _emit_router(nc, pools, state, 0)
    for oc in range(OCCH):
        for k in range(KB):
            for ci in range(CICH):
                nc.sync.dma_start(bank[ci][k][oc][:], bank_ap[ci, k, oc])
    _emit_input_tf(nc, pools, state, 0)
    wdyn_q = {0: {}}
    _emit_mixing_oc(nc, pools, state, 0, a_bc0, 0, wdyn_q[0])
    _emit_mixing_oc(nc, pools, state, 0, a_bc0, 1, wdyn_q[0])
    for b in range(BL):
        a_bc = None
        if b + 1 < BL:
            a_bc = _emit_router(nc, pools, state, b + 1)
            wdyn_q[b + 1] = {}
        _emit_sample_conv(nc, pools, state, b, wdyn_q[b], 0)
        if a_bc is not None:
            # next-sample DVE work goes BETWEEN the halves so it never
            # blocks this sample's PSUM inverse-transforms in the FIFO
            _emit_input_tf(nc, pools, state, b + 1)
            _emit_mixing_oc(nc, pools, state, b + 1, a_bc, 0, wdyn_q[b + 1])
        _emit_sample_conv(nc, pools, state, b, wdyn_q.pop(b), 1)
        if a_bc is not None:
            _emit_mixing_oc(nc, pools, state, b + 1, a_bc, 1, wdyn_q[b + 1])

    for name in ("pyh_psum", "aux_psum", "tft", "ysb", "small", "wdyn",
                 "wacc", "xstage"):
        pools[name].release()
    const_p.release()


_NC_CACHE = {}


def _build():
    nc = bacc.Bacc("TRN2", target_bir_lowering=False, debug=False,
                   enable_asserts=False)
    x_d = nc.dram_tensor("x", [BL, CIN, H, W], F32, kind="ExternalInput")
    bank_d = nc.dram_tensor("bank_t", [CICH, KB, OCCH, 128, NT, 128],
                            BF16, kind="ExternalInput")
    fc1wT_d = nc.dram_tensor("fc1_wT", [128, CICH, HID], F32,
                             kind="ExternalInput")
    fc1b_d = nc.dram_tensor("fc1_b", [HID], F32, kind="ExternalInput")
    fc2wT_d = nc.dram_tensor("fc2_wT", [HID, KB], F32, kind="ExternalInput")
    fc2b_d = nc.dram_tensor("fc2_b", [KB], F32, kind="ExternalInput")
    y_d = nc.dram_tensor("y", [BL, COUT, H, W], F32, kind="ExternalOutput")
    with tile.TileContext(nc) as tc:
        build_kernel(nc, tc, x_d.ap(), bank_d.ap(), fc1wT_d.ap(),
                     fc1b_d.ap(), fc2wT_d.ap(), fc2b_d.ap(), y_d.ap())
    nc.compile()
    return nc


def get_nc():
    if "nc" not in _NC_CACHE:
        _NC_CACHE["nc"] = _build()
    return _NC_CACHE["nc"]


def make_in_maps(x, weight_bank, fc1_w, fc1_b, fc2_w, fc2_b):
    x = np.ascontiguousarray(np.asarray(x, dtype=np.float32))
    wb = np.asarray(weight_bank, np.float32)
    # Winograd G-transform along the H tap axis (dh): 3 -> 4 components
    w0, w1, w2 = wb[..., 0, :], wb[..., 1, :], wb[..., 2, :]
    wg = np.stack([w0, (w0 + w1 + w2) * 0.5, (w0 - w1 + w2) * 0.5, w2],
                  axis=3)                       # [K, Cout, Cin, 4, 3]
    # -> [ciCH, K, ocCH, cin128, (c,dw), cout128] bf16
    wgt = wg.reshape(KB, OCCH, 128, CICH, 128, NT)
    wgt = wgt.transpose(3, 0, 1, 4, 5, 2)
    bank_t = np.ascontiguousarray(wgt.astype(ml_dtypes.bfloat16))
    fc1 = np.asarray(fc1_w, np.float32)
    fc1t = (fc1.T / float(H * W)).reshape(CICH, 128, HID).transpose(1, 0, 2)
    fc1t = np.ascontiguousarray(fc1t)
    fc2t = np.ascontiguousarray(np.asarray(fc2_w, np.float32).T)
    rep = {
        "bank_t": bank_t,
        "fc1_wT": fc1t,
        "fc1_b": np.ascontiguousarray(np.asarray(fc1_b, np.float32)),
        "fc2_wT": fc2t,
        "fc2_b": np.ascontiguousarray(np.asarray(fc2_b, np.float32)),
    }
    return [dict(rep, x=np.ascontiguousarray(x[c * BL:(c + 1) * BL]))
            for c in range(N_CORES)]


def kernel(x, weight_bank, fc1_w, fc1_b, fc2_w, fc2_b):
    nc = get_nc()
    in_maps = make_in_maps(x, weight_bank, fc1_w, fc1_b, fc2_w, fc2_b)
    res = bass_utils.run_bass_kernel_spmd(nc, in_maps,
                                          core_ids=list(range(N_CORES)))
    return np.concatenate([r["y"] for r in res.results], axis=0)
